# revision 29
# baseline (speedup 1.0000x reference)
"""Trainium2 Bass kernel for nn_AdaptiveEMAModel (gated delta-rule EMA memory model).

Data-parallel over batch: 8 cores x 8 batches each. Per core:
  - embedding gather (indirect DMA) -> FFN -> LayerNorm (folded into kp matmul) -> k_all
  - chunked delta-rule scan (C=128) via 1st-order Neumann triangular solve on PE
  - final read @ rp_w.T @ out_w.T projection streamed against out_w.

Self-contained: hardcodes all shapes. kernel(**inputs) takes FULL numpy inputs,
returns FULL [64, 32000] float32 output.
"""
import sys
import numpy as np

sys.path.insert(0, "/opt/trn_rl_repo")

B, L, H, V = 64, 2048, 128, 32000
NCORES = 8
BPC = B // NCORES          # 8 batches per core
P = 128                    # partitions / tile size
NT = L // P                # 16 chunks (tiles) per batch
TILES = BPC * NT           # 128 tiles per core
C_EMA = 1.0 / L            # (1 - alpha)
GATE2 = 0.16               # GATE_THRESH ** 2
EPS_LN = 1e-5
VCHUNK = 500               # out_w streaming chunk (<=512 fp32 psum limit)
NVC = V // VCHUNK          # 64

_COMPILED = {}


def _build(debug=False):
    import concourse.bass as bass
    import concourse.tile as tile
    from concourse import bacc, mybir

    f32 = mybir.dt.float32
    bf16 = mybir.dt.bfloat16
    i32 = mybir.dt.int32
    AX = mybir.AxisListType
    OP = mybir.AluOpType
    AF = mybir.ActivationFunctionType

    nc = bacc.Bacc()

    # ---------------- external inputs (per-core shards / replicated weights)
    idx_ext = nc.declare_dram_parameter("idx", [P, TILES], i32, isOutput=False)
    emb_ext = nc.declare_dram_parameter("embed_w", [V, H], f32, isOutput=False)
    w1T_ext = nc.declare_dram_parameter("w1T", [H, 2 * H], f32, isOutput=False)
    b1_ext = nc.declare_dram_parameter("b1", [P, 2], f32, isOutput=False)
    w2T_ext = nc.declare_dram_parameter("w2T", [P, 2, H], f32, isOutput=False)
    b2_ext = nc.declare_dram_parameter("b2", [H, 1], f32, isOutput=False)
    akpT_ext = nc.declare_dram_parameter("akpT", [H, H], f32, isOutput=False)
    selr_ext = nc.declare_dram_parameter("selr", [3, P], f32, isOutput=False)
    r1T_ext = nc.declare_dram_parameter("r1T", [3, P], f32, isOutput=False)
    ident_ext = nc.declare_dram_parameter("ident", [P, P], f32, isOutput=False)
    ones_ext = nc.declare_dram_parameter("onesc", [P, 1], f32, isOutput=False)
    cmask_ext = nc.declare_dram_parameter("cmask", [P, P], f32, isOutput=False)
    lastm_ext = nc.declare_dram_parameter("lastm", [P, 1], f32, isOutput=False)
    rpwT_ext = nc.declare_dram_parameter("rpwT", [H, H], f32, isOutput=False)
    rpb_ext = nc.declare_dram_parameter("rpb", [H, 1], f32, isOutput=False)
    outwT_ext = nc.declare_dram_parameter("outwT", [H, V], f32, isOutput=False)
    out_ext = nc.declare_dram_parameter("out", [BPC, V], f32, isOutput=True)
    if debug:
        dbgk_ext = nc.declare_dram_parameter("dbg_k", [P, TILES, P], f32, isOutput=True)
        dbgm_ext = nc.declare_dram_parameter("dbg_mt", [H, BPC, H], f32, isOutput=True)
        dbgq_ext = nc.declare_dram_parameter("dbg_q", [H, BPC], f32, isOutput=True)

    with tile.TileContext(nc) as tc:
        import contextlib
        ctx = contextlib.ExitStack()
        with ctx:
            consts = ctx.enter_context(tc.tile_pool(name="consts", bufs=1))
            store = ctx.enter_context(tc.tile_pool(name="store", bufs=1))
            work = ctx.enter_context(tc.tile_pool(name="work", bufs=3))
            smalls = ctx.enter_context(tc.tile_pool(name="smalls", bufs=4))
            gpool = ctx.enter_context(tc.tile_pool(name="gpool", bufs=3))
            wvpool = ctx.enter_context(tc.tile_pool(name="wvpool", bufs=4))
            pps = ctx.enter_context(tc.tile_pool(name="pps", bufs=6, space="PSUM"))
            ptail = ctx.enter_context(tc.tile_pool(name="ptail", bufs=2, space="PSUM"))

            # ---------------- constants to SBUF
            def cload(ext, shape, dtype=f32):
                t = consts.tile(shape, dtype, tag=ext.name)
                nc.sync.dma_start(out=t[:], in_=ext[:])
                return t

            idx_sb = cload(idx_ext, [P, TILES], i32)
            w1T = cload(w1T_ext, [H, 2 * H])
            b1 = cload(b1_ext, [P, 2])
            w2T = cload(w2T_ext, [P, 2, H])
            b2 = cload(b2_ext, [H, 1])
            akpT = cload(akpT_ext, [H, H])
            selr = cload(selr_ext, [3, P])
            r1T = cload(r1T_ext, [3, P])
            ident = cload(ident_ext, [P, P])
            onesc = cload(ones_ext, [P, 1])
            cmask = cload(cmask_ext, [P, P])
            lastm = cload(lastm_ext, [P, 1])
            rpwT = cload(rpwT_ext, [H, H])
            rpb = cload(rpb_ext, [H, 1])
            ident_bf = consts.tile([P, P], bf16)
            nc.vector.tensor_copy(out=ident_bf[:], in_=ident[:])
            eps_ln = consts.tile([P, 1], f32)
            nc.vector.memset(eps_ln[:], EPS_LN)
            eps_nk = consts.tile([P, 1], f32)
            nc.vector.memset(eps_nk[:], 1e-24)

            # ---------------- persistent per-core stores
            Kst = store.tile([P, TILES, P], f32)        # token-major k  [tokmod, tile, h] 8MB
            KnT = store.tile([P, TILES, P], bf16)       # token-major kn (M-update lhsT) 4MB
            KnH = store.tile([P, TILES, P], bf16)       # H-major kn (A/VP lhsT) 4MB
            ntS = store.tile([P, TILES], f32)           # fire threshold 0.16*||k||^2
            rkS = store.tile([P, TILES], f32)           # 1/||k||
            Qall = store.tile([H, BPC], f32)            # q = k[:, -1] per batch
            MTf = store.tile([H, BPC, H], f32)          # M^T accumulator per batch (f32 SBUF)
            nc.vector.memset(MTf[:], 0.0)

            # ================= PREFIX: embedding -> FFN -> LN-folded k =================
            for t in range(TILES):
                b, g = divmod(t, NT)
                # gather 128 token rows: [tok, H]
                htok = gpool.tile([P, H], f32, tag="htok")
                nc.gpsimd.indirect_dma_start(
                    out=htok[:], out_offset=None,
                    in_=emb_ext[:],
                    in_offset=bass.IndirectOffsetOnAxis(ap=idx_sb[:, t:t + 1], axis=0),
                )
                # transpose -> H-major
                ps_h = pps.tile([P, P], f32, tag="ps")
                nc.tensor.transpose(out=ps_h[:], in_=htok[:], identity=ident[:])
                hT = work.tile([H, P], f32, tag="hT")
                nc.scalar.activation(out=hT[:], in_=ps_h[:], func=AF.Copy)

                # FFN mm1 (2x128 out chunks) + relu
                ps_ffa = pps.tile([P, P], f32, tag="ps")
                ps_ffb = pps.tile([P, P], f32, tag="ps")
                nc.tensor.matmul(out=ps_ffa[:], lhsT=w1T[:, 0:H], rhs=hT[:], start=True, stop=True)
                nc.tensor.matmul(out=ps_ffb[:], lhsT=w1T[:, H:2 * H], rhs=hT[:], start=True, stop=True)
                ffa = work.tile([P, P], f32, tag="ffa")
                ffb = work.tile([P, P], f32, tag="ffb")
                nc.scalar.activation(out=ffa[:], in_=ps_ffa[:], func=AF.Relu, bias=b1[:, 0:1])
                nc.scalar.activation(out=ffb[:], in_=ps_ffb[:], func=AF.Relu, bias=b1[:, 1:2])
                # FFN mm2 (accumulate 2 chunks)
                ps_x = pps.tile([P, P], f32, tag="ps")
                nc.tensor.matmul(out=ps_x[:], lhsT=w2T[:, 0, :], rhs=ffa[:], start=True, stop=False)
                nc.tensor.matmul(out=ps_x[:], lhsT=w2T[:, 1, :], rhs=ffb[:], start=False, stop=True)
                xb = work.tile([H, P], f32, tag="xb")
                nc.scalar.activation(out=xb[:], in_=ps_x[:], func=AF.Identity, bias=b2[:])
                x = work.tile([H, P], f32, tag="x")
                nc.vector.tensor_add(out=x[:], in0=xb[:], in1=hT[:])

                # LN stats (token-major cols via lhsT=x)
                x2 = work.tile([H, P], f32, tag="x2")
                nc.vector.tensor_mul(out=x2[:], in0=x[:], in1=x[:])
                ps_st = pps.tile([P, 2], f32, tag="ps")
                nc.tensor.matmul(out=ps_st[:, 0:1], lhsT=x[:], rhs=onesc[:], start=True, stop=True)
                nc.tensor.matmul(out=ps_st[:, 1:2], lhsT=x2[:], rhs=onesc[:], start=True, stop=True)
                mu = smalls.tile([P, 1], f32, tag="mu")
                nc.vector.tensor_scalar_mul(out=mu[:], in0=ps_st[:, 0:1], scalar1=1.0 / H)
                var = smalls.tile([P, 1], f32, tag="var")
                # var = sumsq/H - mu^2
                musq = smalls.tile([P, 1], f32, tag="musq")
                nc.vector.tensor_mul(out=musq[:], in0=mu[:], in1=mu[:])
                nc.vector.tensor_scalar(out=var[:], in0=ps_st[:, 1:2], scalar1=1.0 / H,
                                        scalar2=None, op0=OP.mult)
                nc.vector.tensor_sub(out=var[:], in0=var[:], in1=musq[:])
                # rstd = 1/sqrt(var+eps)
                sd = smalls.tile([P, 1], f32, tag="sd")
                nc.scalar.activation(out=sd[:], in_=var[:], func=AF.Sqrt, bias=eps_ln[:])
                combo = smalls.tile([P, 4], f32, tag="combo")
                nc.vector.reciprocal(out=combo[:, 0:1], in_=sd[:])
                nc.vector.tensor_mul(out=combo[:, 1:2], in0=mu[:], in1=combo[:, 0:1])
                nc.vector.memset(combo[:, 2:3], 1.0)
                # transpose (rstd, s, ones) -> rows
                ps_rs = pps.tile([3, P], f32, tag="ps")
                nc.tensor.transpose(out=ps_rs[:], in_=combo[:, 0:3], identity=ident[:])
                rs3 = smalls.tile([3, P], f32, tag="rs3")
                nc.scalar.activation(out=rs3[:], in_=ps_rs[:], func=AF.Copy)

                # k_pre = A_kp @ x ; Rbc = bcast(rstd); r1 = -c0*s + c1
                ps_kp = pps.tile([P, P], f32, tag="ps")
                nc.tensor.matmul(out=ps_kp[:], lhsT=akpT[:], rhs=x[:], start=True, stop=True)
                ps_rb = pps.tile([P, P], f32, tag="ps")
                nc.tensor.matmul(out=ps_rb[:], lhsT=selr[:], rhs=rs3[:], start=True, stop=True)
                rb_sb = work.tile([P, P], f32, tag="rb")
                nc.scalar.activation(out=rb_sb[:], in_=ps_rb[:], func=AF.Copy)
                ps_r1 = pps.tile([P, P], f32, tag="ps")
                nc.tensor.matmul(out=ps_r1[:], lhsT=r1T[:], rhs=rs3[:], start=True, stop=True)
                k1 = work.tile([H, P], f32, tag="k1")
                nc.vector.tensor_mul(out=k1[:], in0=ps_kp[:], in1=rb_sb[:])
                kh = work.tile([H, P], f32, tag="kh")
                nc.vector.tensor_add(out=kh[:], in0=k1[:], in1=ps_r1[:])

                if g == NT - 1:
                    nc.scalar.activation(out=Qall[:, b:b + 1], in_=kh[:, P - 1:P], func=AF.Copy)

                # ||k||^2 -> rk, nt
                k2 = work.tile([H, P], f32, tag="k2")
                nc.vector.tensor_mul(out=k2[:], in0=kh[:], in1=kh[:])
                ps_nk = pps.tile([P, 2], f32, tag="ps")
                nc.tensor.matmul(out=ps_nk[:, 0:1], lhsT=k2[:], rhs=onesc[:], start=True, stop=True)
                nc.vector.tensor_scalar_mul(out=ntS[:, t:t + 1], in0=ps_nk[:, 0:1], scalar1=GATE2)
                nk_sd = smalls.tile([P, 1], f32, tag="nksd")
                nc.scalar.activation(out=nk_sd[:], in_=ps_nk[:, 0:1], func=AF.Sqrt, bias=eps_nk[:])
                nc.vector.reciprocal(out=rkS[:, t:t + 1], in_=nk_sd[:])

                # token-major K, Kn ; H-major Kn
                ps_kT = pps.tile([P, P], f32, tag="ps")
                nc.tensor.transpose(out=ps_kT[:], in_=kh[:], identity=ident[:])
                nc.scalar.activation(out=Kst[:, t, :], in_=ps_kT[:], func=AF.Copy)
                nc.vector.tensor_scalar_mul(out=KnT[:, t, :], in0=Kst[:, t, :], scalar1=rkS[:, t:t + 1])
                ps_knH = pps.tile([P, P], bf16, tag="ps")
                nc.tensor.transpose(out=ps_knH[:], in_=KnT[:, t, :], identity=ident_bf[:])
                nc.scalar.activation(out=KnH[:, t, :], in_=ps_knH[:], func=AF.Copy)

            # ================= SCAN: 16 sequential chunks x 8 batches =================
            for g in range(NT):
                for b in range(BPC):
                    t = b * NT + g
                    last = (g == NT - 1)
                    # A = Kn^T Kn (psum, [i,j] symmetric)
                    ps_A = pps.tile([P, P], f32, tag="ps")
                    nc.tensor.matmul(out=ps_A[:], lhsT=KnH[:, t, :], rhs=KnH[:, t, :], start=True, stop=True)
                    # Atil = c * A * [j<i] (*fire later)
                    At = wvpool.tile([P, P], f32, tag="At")
                    nc.vector.tensor_mul(out=At[:], in0=ps_A[:], in1=cmask[:])

                    if g == 0:
                        U = Kst[:, t, :]        # alias, no VP
                        fire = None
                    else:
                        mtsb = wvpool.tile([H, H], bf16, tag="mtsb")
                        nc.scalar.activation(out=mtsb[:], in_=MTf[:, b, :], func=AF.Copy)
                        ps_vp = pps.tile([P, P], f32, tag="ps")
                        nc.tensor.matmul(out=ps_vp[:], lhsT=KnH[:, t, :], rhs=mtsb[:], start=True, stop=True)
                        Ut = wvpool.tile([P, P], f32, tag="Ut")
                        nc.vector.tensor_sub(out=Ut[:], in0=Kst[:, t, :], in1=ps_vp[:])
                        U = Ut[:]
                        # fire = (||u||^2 >= 0.16||k||^2)
                        u2 = wvpool.tile([P, P], f32, tag="u2")
                        nc.vector.tensor_mul(out=u2[:], in0=U, in1=U)
                        nu = smalls.tile([P, 1], f32, tag="nu")
                        nc.vector.tensor_reduce(out=nu[:], in_=u2[:], axis=AX.X, op=OP.add)
                        fire = smalls.tile([P, 1], f32, tag="fire")
                        nc.vector.tensor_tensor(out=fire[:], in0=nu[:], in1=ntS[:, t:t + 1], op=OP.is_ge)
                        if last:
                            nc.vector.tensor_mul(out=fire[:], in0=fire[:], in1=lastm[:])
                        nc.vector.tensor_scalar_mul(out=At[:], in0=At[:], scalar1=fire[:])

                    # W1 = Atil @ U ; D = U - W1
                    ps_w1 = pps.tile([P, P], f32, tag="ps")
                    nc.tensor.matmul(out=ps_w1[:], lhsT=At[:], rhs=U, start=True, stop=True)
                    D = wvpool.tile([P, P], f32, tag="D")
                    nc.vector.tensor_sub(out=D[:], in0=U, in1=ps_w1[:])
                    Dt = wvpool.tile([P, P], bf16, tag="Dt")
                    if g == 0:
                        sc = lastm[:] if last else None
                        if sc is None:
                            nc.vector.tensor_scalar_mul(out=Dt[:], in0=D[:], scalar1=C_EMA)
                        else:
                            nc.vector.tensor_scalar(out=Dt[:], in0=D[:], scalar1=C_EMA,
                                                    scalar2=sc, op0=OP.mult, op1=OP.mult)
                    else:
                        cf = smalls.tile([P, 1], f32, tag="cf")
                        nc.vector.tensor_scalar_mul(out=cf[:], in0=fire[:], scalar1=C_EMA)
                        nc.vector.tensor_scalar_mul(out=Dt[:], in0=D[:], scalar1=cf[:])
                    # M^T += Kn^T(tm) @ Dt  (psum per chunk, accumulate in SBUF f32)
                    ps_up = pps.tile([P, P], f32, tag="ps")
                    nc.tensor.matmul(out=ps_up[:], lhsT=KnT[:, t, :], rhs=Dt[:],
                                     start=True, stop=True)
                    nc.vector.tensor_add(out=MTf[:, b, :], in0=MTf[:, b, :], in1=ps_up[:])

            # ================= TAIL: read -> rp -> out_w =================
            ps_rd = ptail.tile([H, BPC], f32, tag="tail")
            for b in range(BPC):
                nc.tensor.matmul(out=ps_rd[:, b:b + 1], lhsT=MTf[:, b, :], rhs=Qall[:, b:b + 1],
                                 start=True, stop=True, skip_group_check=True)
            read_sb = smalls.tile([H, BPC], f32, tag="readsb")
            nc.scalar.activation(out=read_sb[:], in_=ps_rd[:], func=AF.Copy)
            ps_z = ptail.tile([H, BPC], f32, tag="tail")
            nc.tensor.matmul(out=ps_z[:], lhsT=rpwT[:], rhs=read_sb[:], start=True, stop=True)
            z_sb = smalls.tile([H, BPC], f32, tag="zsb")
            nc.scalar.activation(out=z_sb[:], in_=ps_z[:], func=AF.Identity, bias=rpb[:])

            if debug:
                nc.sync.dma_start(out=dbgk_ext[:], in_=Kst[:])
                nc.sync.dma_start(out=dbgm_ext[:], in_=MTf[:])
                nc.sync.dma_start(out=dbgq_ext[:], in_=Qall[:])

            for vc in range(NVC):
                wv = wvpool.tile([H, VCHUNK], f32, tag="wv")
                nc.sync.dma_start(out=wv[:], in_=outwT_ext[:, vc * VCHUNK:(vc + 1) * VCHUNK])
                ps_lo = ptail.tile([BPC, VCHUNK], f32, tag="tail")
                nc.tensor.matmul(out=ps_lo[:], lhsT=z_sb[:], rhs=wv[:], start=True, stop=True)
                lo_sb = wvpool.tile([BPC, VCHUNK], f32, tag="losb")
                nc.vector.tensor_copy(out=lo_sb[:], in_=ps_lo[:])
                nc.sync.dma_start(out=out_ext[:, vc * VCHUNK:(vc + 1) * VCHUNK], in_=lo_sb[:])

    nc.compile()
    return nc


def _prep_shards(inputs):
    seq = np.asarray(inputs["seq"]).astype(np.int32)         # [B, L]
    embed_w = np.ascontiguousarray(np.asarray(inputs["embed_w"], np.float32))
    ff_w1 = np.asarray(inputs["ff_w1"], np.float32)
    ff_b1 = np.asarray(inputs["ff_b1"], np.float32)
    ff_w2 = np.asarray(inputs["ff_w2"], np.float32)
    ff_b2 = np.asarray(inputs["ff_b2"], np.float32)
    ln_g = np.asarray(inputs["ln_g"], np.float32)
    ln_b = np.asarray(inputs["ln_b"], np.float32)
    kp_w = np.asarray(inputs["kp_w"], np.float32)
    rp_w = np.asarray(inputs["rp_w"], np.float32)
    rp_b = np.asarray(inputs["rp_b"], np.float32)
    out_w = np.asarray(inputs["out_w"], np.float32)

    A = kp_w * ln_g[None, :]
    c0 = A.sum(1)
    c1 = kp_w @ ln_b
    selr = np.zeros((3, P), np.float32); selr[0, :] = 1.0
    r1T = np.zeros((3, P), np.float32); r1T[1, :] = -c0; r1T[2, :] = c1
    cmask = (C_EMA * np.tril(np.ones((P, P), np.float32), -1).T)  # [j, i] = c if j < i
    lastm = np.ones((P, 1), np.float32); lastm[P - 1, 0] = 0.0
    common = {
        "embed_w": embed_w,
        "w1T": np.ascontiguousarray(ff_w1.T),
        "b1": np.ascontiguousarray(ff_b1.reshape(2, P).T),
        "w2T": np.ascontiguousarray(ff_w2.T.reshape(2, P, H).transpose(1, 0, 2)),
        "b2": np.ascontiguousarray(ff_b2[:, None]),
        "akpT": np.ascontiguousarray(A.T),
        "selr": selr, "r1T": r1T,
        "ident": np.eye(P, dtype=np.float32),
        "onesc": np.ones((P, 1), np.float32),
        "cmask": np.ascontiguousarray(cmask),
        "lastm": lastm,
        "rpwT": np.ascontiguousarray(rp_w.T),
        "rpb": np.ascontiguousarray(rp_b[:, None]),
        "outwT": np.ascontiguousarray(out_w.T),
    }
    in_maps = []
    for c in range(NCORES):
        sl = seq[c * BPC:(c + 1) * BPC]                       # [8, 2048]
        # idx[p, b*NT+g] = seq[b, g*128+p]
        idx = np.ascontiguousarray(sl.reshape(BPC, NT, P).transpose(2, 0, 1).reshape(P, TILES))
        m = dict(common)
        m["idx"] = idx
        in_maps.append(m)
    return in_maps


def kernel(**inputs):
    from concourse.bass_utils import run_bass_kernel_spmd

    if "nc" not in _COMPILED:
        _COMPILED["nc"] = _build()
    nc = _COMPILED["nc"]
    in_maps = _prep_shards(inputs)
    res = run_bass_kernel_spmd(nc, in_maps, core_ids=list(range(NCORES)))
    out = np.concatenate([r["out"] for r in res.results], axis=0)
    out = out + np.asarray(inputs["out_b"], np.float32)[None, :]
    return out.astype(np.float32)


def run_traced(inputs):
    """Run with NTFF tracing; returns BassKernelResults (exec_time_ns etc.)."""
    from concourse.bass_utils import run_bass_kernel_spmd

    if "nc" not in _COMPILED:
        _COMPILED["nc"] = _build()
    nc = _COMPILED["nc"]
    in_maps = _prep_shards(inputs)
    return run_bass_kernel_spmd(nc, in_maps, core_ids=list(range(NCORES)), trace=True)


def run_debug(inputs):
    from concourse.bass_utils import run_bass_kernel_spmd

    nc = _build(debug=True)
    in_maps = _prep_shards(inputs)
    res = run_bass_kernel_spmd(nc, in_maps, core_ids=list(range(NCORES)))
    return res.results


if __name__ == "__main__":
    nc = _build()
    print("build ok", len(nc.m.functions[0].allocations))
